# revision 7
# baseline (speedup 1.0000x reference)
"""Trainium2 Bass kernel for nn_CustomLoss_90537910600076 (nms_detection).

Computes, for in_signal/ref_signal [2048, 4096] f32:
  [total_loss, cosine_similarity, p2p_loss, mse_loss]  (f32 [4])

Pure data parallel over the batch dim across 8 NeuronCores (256 rows per
core, 2 blocks of 128 partitions).

v2 design (pair-space, fp16):
  The host casts inputs to f16 and deinterleaves even/odd positions into
  [rows, 2, 2048] planes, halving HBM traffic and making every sliding-max
  cascade op packed 2-byte (DVE 2x_1p mode, 2 elem/cycle).

  Pair space: pair u = positions (2u, 2u+1). With XE[u]=x[2u], XO[u]=x[2u+1]:
    p  = max(XE, XO)                       (pair max)
    d1 = max(p[i], p[i+1]); m2 = max(d1[i], d1[i+2]); m4 = max(m2[i], m2[i+4])
    M4[u] = max(m4[u-4], p[u+4])           = max over pairs u-4..u+4
    T1[u] = max(M4[u], XO[u-5]) = W19[2u]  (19-window max at even pos)
    T2[u] = max(M4[u], XE[u+5]) = W19[2u+1]
    U1[u] = max(T1[u-5], T1[u+5])          (39-window minus self at even pos)
    U2[u] = max(T2[u-5], T2[u+5])
  d10 peak (= x >= pooled19; strict-local-max is implied up to exact ties,
  which are negligible): pk_e = XE where XE >= T1 else 0, pk_o likewise.
  d20 count: #(XE >= U1) + #(XO > U2)  (GE/GT split breaks pair ties).
  Edge positions 0 and L-1 (never peaks in the reference) are excluded by
  poisoning T/U there with +BIG after U is built.

  Per-row sufficient stats (8 cols): dot, na2, nb2, cntE_in, cntO_in,
  cntE_ref, cntO_ref, p2p. Host combines: mse via na2+nb2-2dot identity.

Engine split per block: Pool does pad memsets + p + d1 + poisons; DVE does
m2/m4/M4/T12/U12 (f16 packed, 2x) and the custom-op compares/reductions
(PK select, CNT_GE/CNT_GT counts, SQDS, TENSOR_TENSOR_REDUCE dot); ACT does
the two Square-accumulate passes (na2/nb2).
"""

import sys

if "/opt/trn_rl_repo" not in sys.path:
    sys.path.insert(0, "/opt/trn_rl_repo")

import numpy as np

B, L = 2048, 4096
NCORES = 8
ROWS_PER_CORE = B // NCORES      # 256
NBLK = ROWS_PER_CORE // 128      # 2
NHALF = L // 2                   # 2048 pairs per row
PADP = 12                        # pair-space pad each side
NP = NHALF + 2 * PADP            # 2072
ALPHA, BETA = 1.0, 0.5
NEG = -60000.0                   # -inf stand-in within f16 range
BIG = 60000.0

_CACHE = {}


def _mkap(bass, t, col_off, dims):
    """Custom view of a tile AP `t` ([128, ...]): keep the partition dim,
    replace free dims with explicit [step, count] pairs (element units),
    offset by col_off elements from t's start."""
    part = [list(d) for d in t.ap][0]
    return bass.AP(
        tensor=t.tensor,
        offset=int(t.offset) + int(col_off),
        ap=[part] + [[int(s), int(c)] for s, c in dims],
    )


def _register_custom_ops():
    """Define + self-pin the fused DVE ops, append them to dve_ops.OPS."""
    if "ops" in _CACHE:
        return _CACHE["ops"]
    import concourse.dve_ops as dve_ops
    from concourse.dve_spec import (
        Spec, Src0, Src1, C0, Zero, lower, select, sq, ne, _has_src1,
    )
    from concourse.dve_uop import DveOpSpec
    from operator import add as _add

    def _flat2(in0, in1):
        a = np.asarray(in0).reshape(np.asarray(in0).shape[0], -1)
        bb = np.asarray(in1).reshape(np.asarray(in1).shape[0], -1)
        return a, bb

    def _ref_pk(in0, in1, s0, s1, imm2):
        a, bb = _flat2(in0, in1)
        return np.where(a >= bb, a, np.float32(0.0)).astype(np.float32)

    def _ref_cnt(in0, in1, s0, s1, imm2):
        a, bb = _flat2(in0, in1)
        b = ((a >= bb) & (a != 0.0)).astype(np.float32)
        return b, s0 + b.sum(axis=-1, keepdims=True)

    def _ref_cntgt(in0, in1, s0, s1, imm2):
        a, bb = _flat2(in0, in1)
        b = ((a > bb) & (a != 0.0)).astype(np.float32)
        return b, s0 + b.sum(axis=-1, keepdims=True)

    def _ref_sqds(in0, in1, s0, s1, imm2):
        a, bb = _flat2(in0, in1)
        b = ((a.astype(np.float32) - bb) ** 2).astype(np.float32)
        return b, s0 + b.sum(axis=-1, keepdims=True)

    specs = [
        ("ANT_NMS_PK", Spec(body=select(Src0 >= Src1, Src0, Zero), reference=_ref_pk)),
        (
            "ANT_NMS_CNT",
            Spec(
                body=(Src0 >= Src1) & ne(Src0, Zero),
                accum=_add,
                accum_init=C0,
                reference=_ref_cnt,
            ),
        ),
        (
            "ANT_NMS_CNTGT",
            Spec(
                body=(Src0 > Src1) & ne(Src0, Zero),
                accum=_add,
                accum_init=C0,
                reference=_ref_cntgt,
            ),
        ),
        (
            "ANT_NMS_SQDS",
            Spec(
                body=sq(Src0 - Src1),
                accum=_add,
                accum_init=C0,
                reference=_ref_sqds,
            ),
        ),
    ]

    ops = {}
    for i, (name, spec) in enumerate(specs):
        if any(op.name == name for op in dve_ops.OPS):
            ops[name] = next(op for op in dve_ops.OPS if op.name == name)
            continue
        shas = {}
        for ver in ("v3", "v4"):
            r = DveOpSpec(
                name=name, opcode=0, uops=lower(spec, ver=ver),
                rd1_en=_has_src1(spec),
            )
            shas[ver] = r.sha(ver)
        op = dve_ops.DveOp(name, spec, subdim=False, uops_sha=shas)
        dve_ops.OPS.append(op)
        dve_ops.CUSTOM_DVE_SPECS[name] = spec
        ops[name] = op
    dve_ops._SUB_OPCODE_FOR_NAME = {
        op.name: dve_ops._CUSTOM_DVE_ROW_BASE + i for i, op in enumerate(dve_ops.OPS)
    }
    assert max(dve_ops._SUB_OPCODE_FOR_NAME.values()) < 0x20
    _CACHE["ops"] = ops
    return ops


def _build(repeat=1):
    """Build the SPMD program. `repeat` unrolls the whole 2-block body N
    times inside one NEFF (benchmarking only; outputs are just rewritten)."""
    import concourse.bass as bass
    import concourse.bacc as bacc
    import concourse.tile as tile
    import concourse.mybir as mybir
    from contextlib import ExitStack

    ops = _register_custom_ops()
    OP_PK, OP_CNT, OP_CNTGT, OP_SQDS = (
        ops["ANT_NMS_PK"], ops["ANT_NMS_CNT"], ops["ANT_NMS_CNTGT"],
        ops["ANT_NMS_SQDS"],
    )
    from concourse.dve_ops import TENSOR_TENSOR_REDUCE as OP_TTR

    f16 = mybir.dt.float16
    f32 = mybir.dt.float32
    Alu = mybir.AluOpType
    Act = mybir.ActivationFunctionType

    nc = bacc.Bacc("TRN2", target_bir_lowering=False)
    x_in = nc.dram_tensor(
        "x_in", [ROWS_PER_CORE, 2, NHALF], f16, kind="ExternalInput"
    ).ap()
    x_ref = nc.dram_tensor(
        "x_ref", [ROWS_PER_CORE, 2, NHALF], f16, kind="ExternalInput"
    ).ap()
    # one output slice per unrolled repeat so benchmark repeats are never
    # dead code (all but the last would otherwise be eliminated)
    out_stats = nc.dram_tensor(
        "stats_out", [repeat * NBLK, 128, 8], f32, kind="ExternalOutput"
    ).ap()

    with ExitStack() as ctx:
        tc = ctx.enter_context(tile.TileContext(nc))
        sb = ctx.enter_context(tc.tile_pool(name="sb", bufs=1))

        D = NHALF          # valid pair count
        PP = PADP          # data starts at pair index PP

        for rep_b in range(repeat * NBLK):
            b = rep_b % NBLK
            rows = slice(b * 128, (b + 1) * 128)

            XI = sb.tile([128, 2, NP], f16, tag="XI", bufs=2, name=f"XI{rep_b}")
            XR = sb.tile([128, 2, NP], f16, tag="XR", bufs=2, name=f"XR{rep_b}")
            PA = sb.tile([128, NP], f16, tag="PA", bufs=2, name=f"PA{rep_b}")
            PB = sb.tile([128, NP], f16, tag="PB", bufs=2, name=f"PB{rep_b}")
            PC = sb.tile([128, NP], f16, tag="PC", bufs=2, name=f"PC{rep_b}")
            T12 = sb.tile([128, 2, NP], f16, tag="T12", bufs=2, name=f"T{rep_b}")
            U12 = sb.tile([128, 2, NP], f16, tag="U12", bufs=2, name=f"U{rep_b}")
            PKI = sb.tile([128, 2, D], f16, tag="PKI", name=f"PKI{rep_b}")
            PKR = sb.tile([128, 2, D], f16, tag="PKR", name=f"PKR{rep_b}")
            SQ = sb.tile([128, 2, D], f16, tag="SQ", name=f"SQ{rep_b}")
            ACTS = sb.tile([128, 2, D], f16, tag="ACTS", name=f"AS{rep_b}")
            STATS = sb.tile([128, 8], f32, tag="STATS", name=f"ST{rep_b}")

            nc.sync.dma_start(out=XI[:, :, PP : PP + D], in_=x_in[rows, :, :])
            nc.sync.dma_start(out=XR[:, :, PP : PP + D], in_=x_ref[rows, :, :])

            def tmax(eng, out, i0, i1):
                eng.tensor_tensor(out=out, in0=i0, in1=i1, op=Alu.max)

            for t, X in ((0, XI), (1, XR)):
                xh = int(X.ap[1][0])     # parity-plane stride (elements)
                th = int(T12.ap[1][0])
                uh = int(U12.ap[1][0])

                # -inf pads so truncated edge windows are correct
                nc.gpsimd.memset(X[:, :, 0:PP], NEG)
                nc.gpsimd.memset(X[:, :, NP - PADP : NP], NEG)

                # p[i] = max(XE[i], XO[i]) over full padded range
                tmax(
                    nc.vector,
                    PA[:, 0:NP],
                    _mkap(bass, X, 0, [[1, NP]]),
                    _mkap(bass, X, xh, [[1, NP]]),
                )
                # d1[i] = max(p[i], p[i+1])
                tmax(nc.vector, PB[:, 0 : NP - 1], PA[:, 0 : NP - 1], PA[:, 1:NP])
                # m2[i] = max(d1[i], d1[i+2])   covers p[i..i+3]
                tmax(nc.vector, PC[:, 0 : NP - 3], PB[:, 0 : NP - 3], PB[:, 2 : NP - 1])
                # m4[i] = max(m2[i], m2[i+4])   covers p[i..i+7]  (overwrites d1)
                tmax(nc.vector, PB[:, 0 : NP - 7], PC[:, 0 : NP - 7], PC[:, 4 : NP - 3])
                # M4[u] = max(m4[u-4], p[u+4])  covers p[u-4..u+4] (overwrites m2)
                tmax(
                    nc.vector,
                    PC[:, 4 : NP - 7],
                    PB[:, 0 : NP - 11],
                    PA[:, 8 : NP - 3],
                )
                # T1[u] = max(M4[u], XO[u-5]); T2[u] = max(M4[u], XE[u+5])
                # one op over both parities: in1 outer step jumps XO[u-5]->XE[u+5]
                nT = NP - 14  # u in [7, NP-7): M4 valid [4, NP-7)
                tmax(
                    nc.vector,
                    _mkap(bass, T12, 7, [[th, 2], [1, nT]]),
                    _mkap(bass, PC, 7, [[0, 2], [1, nT]]),
                    _mkap(bass, X, xh + 2, [[(0 + 12) - (xh + 2), 2], [1, nT]]),
                )
                # U1[u] = max(T1[u-5], T1[u+5]); U2 likewise
                tmax(
                    nc.vector,
                    _mkap(bass, U12, PP, [[uh, 2], [1, D]]),
                    _mkap(bass, T12, PP - 5, [[th, 2], [1, D]]),
                    _mkap(bass, T12, PP + 5, [[th, 2], [1, D]]),
                )
                # exclude positions 0 and L-1 (never peaks in the reference):
                # poison T (for pk) and U (for count) there, after U is built
                nc.gpsimd.memset(
                    _mkap(bass, T12, PP, [[th + D - 1, 2], [1, 1]]), BIG
                )
                nc.gpsimd.memset(
                    _mkap(bass, U12, PP, [[uh + D - 1, 2], [1, 1]]), BIG
                )

                # pk values (both parities, one op)
                PK = PKI if t == 0 else PKR
                nc.vector._custom_dve(
                    OP_PK,
                    out=PK[:, :, :],
                    in0=_mkap(bass, X, PP, [[xh, 2], [1, D]]),
                    in1=_mkap(bass, T12, PP, [[th, 2], [1, D]]),
                )
                # d20 counts: even GE, odd GT (tie-break)
                nc.vector._custom_dve(
                    OP_CNT,
                    out=SQ[:, 0, 0:D],
                    in0=_mkap(bass, X, PP, [[1, D]]),
                    in1=_mkap(bass, U12, PP, [[1, D]]),
                    s0=0.0,
                    accum_out=STATS[:, 3 + 2 * t : 4 + 2 * t],
                )
                nc.vector._custom_dve(
                    OP_CNTGT,
                    out=SQ[:, 1, 0:D],
                    in0=_mkap(bass, X, xh + PP, [[1, D]]),
                    in1=_mkap(bass, U12, uh + PP, [[1, D]]),
                    s0=0.0,
                    accum_out=STATS[:, 4 + 2 * t : 5 + 2 * t],
                )
                # sum of squares -> na2/nb2  (ACT)
                nc.scalar.activation(
                    out=ACTS[:, :, :],
                    in_=_mkap(bass, X, PP, [[xh, 2], [1, D]]),
                    func=Act.Square,
                    accum_out=STATS[:, 1 + t : 2 + t],
                )

            xih = int(XI.ap[1][0])
            xrh = int(XR.ap[1][0])
            # dot = sum(in*ref)
            nc.vector._custom_dve(
                OP_TTR,
                out=SQ[:, :, :],
                in0=_mkap(bass, XI, PP, [[xih, 2], [1, D]]),
                in1=_mkap(bass, XR, PP, [[xrh, 2], [1, D]]),
                s0=0.0,
                s1=1.0,
                accum_out=STATS[:, 0:1],
            )
            # p2p = sum((pk_in - pk_ref)^2)
            nc.vector._custom_dve(
                OP_SQDS,
                out=SQ[:, :, :],
                in0=PKI[:, :, :],
                in1=PKR[:, :, :],
                s0=0.0,
                accum_out=STATS[:, 7:8],
            )

            nc.sync.dma_start(out=out_stats[rep_b, :, :], in_=STATS[:, 0:8])

    nc.compile()
    return nc


def _get_nc():
    if "nc" not in _CACHE:
        _CACHE["nc"] = _build()
    return _CACHE["nc"]


def _prep(x):
    """[rows, L] f32 -> [rows, 2, NHALF] f16 even/odd deinterleaved."""
    x = np.asarray(x)
    out = np.empty((x.shape[0], 2, NHALF), dtype=np.float16)
    out[:, 0, :] = x[:, 0::2]
    out[:, 1, :] = x[:, 1::2]
    return out


def prep_core_inputs(in_signal, ref_signal, core):
    r = slice(core * ROWS_PER_CORE, (core + 1) * ROWS_PER_CORE)
    return {
        "x_in": _prep(in_signal[r]),
        "x_ref": _prep(ref_signal[r]),
    }


def run_device(in_signal, ref_signal):
    """Run the SPMD kernel; returns per-row stats [B, 8] float32."""
    from concourse.bass_utils import run_bass_kernel_spmd

    nc = _get_nc()
    in_maps = [
        prep_core_inputs(in_signal, ref_signal, c) for c in range(NCORES)
    ]
    res = run_bass_kernel_spmd(nc, in_maps, list(range(NCORES))).results
    stats = np.concatenate(
        [np.asarray(res[c]["stats_out"]).reshape(ROWS_PER_CORE, 8) for c in range(NCORES)],
        axis=0,
    )
    return stats


def finalize(stats):
    """Host combine of per-row stats -> [4] f32 output."""
    dot = stats[:, 0].astype(np.float64)
    na2 = stats[:, 1].astype(np.float64)
    nb2 = stats[:, 2].astype(np.float64)
    n_in = stats[:, 3] + stats[:, 4]
    n_ref = stats[:, 5] + stats[:, 6]
    p2p_sum = stats[:, 7].astype(np.float64)

    sqsum = na2 + nb2 - 2.0 * dot
    mse_i = sqsum / L
    mse_loss = sqsum.sum() / (B * L)
    cosine = (dot / np.sqrt(na2 * nb2)).mean()
    p2p_i = p2p_sum / L
    p2p_loss = p2p_i.sum()
    custom = np.where(n_in != n_ref, mse_i * ALPHA, p2p_i * BETA).sum()
    total = mse_loss + custom
    return np.array([total, cosine, p2p_loss, mse_loss], dtype=np.float32)


def kernel(in_signal, ref_signal):
    stats = run_device(np.asarray(in_signal), np.asarray(ref_signal))
    return finalize(stats)
